# revision 8
# baseline (speedup 1.0000x reference)
"""Trainium2 Bass kernel for nn_HRRAdaptedAttention (B=2, S=8192, D=1024).

out = output + gate * irfft(cumsum_s(rfft(k)*rfft(v)) * conj(rfft(q))),
q/k/v = hidden @ W.T + b.

Sharding: (batch, seq) -> 8 chunks of 2048 positions, one per core.
The rfft/irfft are folded into the projection weights on the host
(fk = h @ (Wk.T @ C) etc.), so everything on device is fp32r matmuls,
elementwise complex arithmetic, and a per-frequency cumsum over the
sequence axis (tensor_tensor_scan, [freq->partitions, seq->free] layout).

Launch 1 (per core): h^T -> fk,fv -> kv = fk*fv -> kv chunk to DRAM,
plus per-frequency chunk totals.
Host: 8x[1025] exclusive prefix over chunk totals (causal carry).
Launch 2: scan(kv, initial=prefix) -> mem; fq; Z = mem*conj(fq);
values = Z @ [A;B] (gate folded); res = output + values.
"""

import numpy as np

B, S, D = 2, 8192, 1024
F = 513
NCORES = 8
CHUNK = 2048
PANEL = 512
NPANEL = CHUNK // PANEL
FT = 4                   # 128-row freq tiles f=0..511; f=512 handled apart
NDP = 8
KVROWS = 1152            # kv dram rows: 512 re + 512 im + kvnyq + fqnyq

_cache = {}


def _host_constants(Wq, bq, Wk, bk, Wv, bv, gate):
    d = np.arange(D, dtype=np.float64)
    f = np.arange(F, dtype=np.float64)
    ang = 2.0 * np.pi * np.outer(d, f) / D
    C = np.cos(ang)
    Sm = -np.sin(ang)

    def fold(W, sign_s=1.0):
        Wt = W.T.astype(np.float64)
        return (Wt @ C).astype(np.float32), (sign_s * (Wt @ Sm)).astype(np.float32)

    MkC, MkS = fold(Wk)
    MvC, MvS = fold(Wv)
    MqC, MqS = fold(Wq, sign_s=-1.0)          # conj(fq) folded

    g = float(np.asarray(gate).reshape(-1)[0])
    w = np.full(F, 2.0)
    w[0] = 1.0
    w[512] = 1.0
    scale = (w * g / D)[:, None]
    A = (scale * C.T).astype(np.float32)       # [F, D] coeff for Zre
    Bm = (scale * Sm.T).astype(np.float32)     # [F, D] coeff for Zim

    bk64, bv64, bq64 = (x.astype(np.float64) for x in (bk, bv, bq))
    bias = np.zeros((6, 520), dtype=np.float32)
    bias[0, :F] = (bk64 @ C).astype(np.float32)
    bias[1, :F] = (bk64 @ Sm).astype(np.float32)
    bias[2, :F] = (bv64 @ C).astype(np.float32)
    bias[3, :F] = (bv64 @ Sm).astype(np.float32)
    bias[4, :F] = (bq64 @ C).astype(np.float32)
    bias[5, :F] = (-(bq64 @ Sm)).astype(np.float32)
    return dict(MkC=MkC, MkS=MkS, MvC=MvC, MvS=MvS, MqC=MqC, MqS=MqS,
                A=A, Bm=Bm, bias=bias)


_WAIT_EXEMPT = {
    "InstNoOp", "InstEventSemaphore", "InstUnconditionalBranch",
    "InstRegisterMove", "InstCall", "InstISA",
}


def _legalize_waits(nc, max_waits=1):
    """TRN2 instruction structs hold one sync-wait command; move extra waits
    onto same-engine nops inserted just before the instruction."""
    import bass_rust
    import concourse.mybir as mybir
    ctr = 0
    for fn in nc.m.functions:
        for blk in fn.blocks:
            new = []
            for inst in blk.instructions:
                if (type(inst).__name__ not in _WAIT_EXEMPT
                        and inst.sync_info is not None):
                    waits = list(inst.sync_info.on_wait)
                    if len(waits) > max_waits:
                        for w in waits[:-max_waits]:
                            nop = mybir.InstNoOp(
                                name=f"I-lglnop-{ctr}", ins=[], outs=[])
                            ctr += 1
                            nop.engine = inst.engine
                            nop.sync_info = bass_rust.SyncInfo(
                                on_wait=[w], on_update=[])
                            new.append(nop)
                        inst.sync_info = bass_rust.SyncInfo(
                            on_wait=waits[-max_waits:],
                            on_update=inst.sync_info.on_update)
                new.append(inst)
            blk.instructions = new


def _make_ht(nc, tc, htp, hnp, pst, h_d, identr, p0):
    """Load h[p0:p0+PANEL] and emit h^T tiles [128d, PANEL] (fp32r)."""
    import concourse.mybir as mybir
    F32R = mybir.dt.float32r
    ht = [htp.tile([128, PANEL], F32R, tag=f"ht_{dp}", name=f"ht_{dp}") for dp in range(NDP)]
    for st in range(PANEL // 128):
        hn = hnp.tile([128, D], F32R, tag="hn")
        nc.sync.dma_start(hn[:], h_d.ap()[p0 + st * 128:p0 + (st + 1) * 128, :])
        for dp in range(NDP):
            tp = pst.tile([128, 128], F32R, tag="trps")
            nc.tensor.transpose(tp[:], hn[:, dp * 128:(dp + 1) * 128],
                                identr[:])
            nc.scalar.copy(ht[dp][:, st * 128:(st + 1) * 128], tp[:])
    return ht


def _build_a(has_bias):
    import concourse.bass as bass
    import concourse.mybir as mybir
    import concourse.tile as tile
    F32, F32R = mybir.dt.float32, mybir.dt.float32r
    AT = mybir.AluOpType

    nc = bass.Bass("TRN2", target_bir_lowering=False, debug=False,
                   num_devices=NCORES)
    h_d = nc.dram_tensor("h", [CHUNK, D], F32R, kind="ExternalInput")
    m_d = {nm: nc.dram_tensor(nm, [D, 512], F32R, kind="ExternalInput")
           for nm in ("MkC", "MkS", "MvC", "MvS")}
    mnyq_d = nc.dram_tensor("Mnyq", [D, 2], F32R, kind="ExternalInput")
    identr_d = nc.dram_tensor("identr", [128, 128], F32R, kind="ExternalInput")
    if has_bias:
        bias_d = nc.dram_tensor("biasA", [1, 4 * 520 + 2], F32R,
                                kind="ExternalInput")
        ones_d = nc.dram_tensor("ones", [1, PANEL], F32R, kind="ExternalInput")
    kvd = nc.dram_tensor("kvd", [KVROWS, CHUNK], F32, kind="ExternalOutput")
    tot_d = nc.dram_tensor("totals", [1056, 1], F32, kind="ExternalOutput")

    with tile.TileContext(nc) as tc:
        with (
            tc.tile_pool(name="const", bufs=1) as cp,
            tc.tile_pool(name="wpool", bufs=1) as wp,
            tc.tile_pool(name="ht", bufs=1) as htp,
            tc.tile_pool(name="hnat", bufs=2) as hnp,
            tc.tile_pool(name="work", bufs=3) as wkp,
            tc.tile_pool(name="acc", bufs=1) as accp,
            tc.tile_pool(name="psA", bufs=3, space="PSUM") as psA,
            tc.tile_pool(name="psN", bufs=1, space="PSUM") as psN,
            tc.tile_pool(name="psT", bufs=2, space="PSUM") as pst,
        ):
            identr = cp.tile([128, 128], F32R, tag="identr")
            nc.sync.dma_start(identr[:], identr_d.ap())
            if has_bias:
                bias = cp.tile([1, 4 * 520 + 2], F32R, tag="bias")
                nc.sync.dma_start(bias[:], bias_d.ap())
                ones = cp.tile([1, PANEL], F32R, tag="ones")
                nc.sync.dma_start(ones[:], ones_d.ap())
            mkv = {}
            for nm in ("MkC", "MkS", "MvC", "MvS"):
                for dp in range(NDP):
                    t = wp.tile([128, 512], F32R, tag=f"m_{nm}_{dp}")
                    nc.sync.dma_start(
                        t[:], m_d[nm].ap()[dp * 128:(dp + 1) * 128, :])
                    mkv[(nm, dp)] = t
            mnyq = []
            for dp in range(NDP):
                t = cp.tile([128, 2], F32R, tag=f"mnyq_{dp}")
                nc.sync.dma_start(t[:], mnyq_d.ap()[dp * 128:(dp + 1) * 128, :])
                mnyq.append(t)

            acc = {i: accp.tile([128, 1], F32, tag=f"acc_{i}", name=f"acc_{i}") for i in range(8)}
            accn = accp.tile([1, 1], F32, tag="acc_n")

            for p in range(NPANEL):
                p0 = p * PANEL
                ht = _make_ht(nc, tc, htp, hnp, pst, h_d, identr, p0)
                for ft in range(FT):
                    ps = {}
                    for i, nm in enumerate(("MkC", "MkS", "MvC", "MvS")):
                        pt = psA.tile([128, PANEL], F32, tag="fwd")
                        for dp in range(NDP):
                            nc.tensor.matmul(
                                pt[:], mkv[(nm, dp)][:, ft * 128:(ft + 1) * 128],
                                ht[dp][:], start=(dp == 0),
                                stop=(dp == NDP - 1 and not has_bias))
                        if has_bias:
                            nc.tensor.matmul(
                                pt[:],
                                bias[:, i * 520 + ft * 128:i * 520 + (ft + 1) * 128],
                                ones[:], start=False, stop=True)
                        ps[nm] = pt
                    fkre = wkp.tile([128, PANEL], F32, tag="fkre")
                    fkim = wkp.tile([128, PANEL], F32, tag="fkim")
                    nc.scalar.copy(fkre[:], ps["MkC"][:])
                    nc.scalar.copy(fkim[:], ps["MkS"][:])
                    t1 = wkp.tile([128, PANEL], F32, tag="t1")
                    t2 = wkp.tile([128, PANEL], F32, tag="t2")
                    kvre = wkp.tile([128, PANEL], F32, tag="kvre")
                    kvim = wkp.tile([128, PANEL], F32, tag="kvim")
                    nc.vector.tensor_tensor(t1[:], fkre[:], ps["MvC"][:],
                                            op=AT.mult)
                    nc.vector.tensor_tensor(t2[:], fkim[:], ps["MvS"][:],
                                            op=AT.mult)
                    nc.vector.tensor_tensor(kvre[:], t1[:], t2[:],
                                            op=AT.subtract)
                    nc.vector.tensor_tensor(t1[:], fkre[:], ps["MvS"][:],
                                            op=AT.mult)
                    nc.vector.tensor_tensor(t2[:], fkim[:], ps["MvC"][:],
                                            op=AT.mult)
                    nc.vector.tensor_tensor(kvim[:], t1[:], t2[:], op=AT.add)
                    nc.sync.dma_start(
                        kvd.ap()[ft * 128:(ft + 1) * 128, p0:p0 + PANEL],
                        kvre[:])
                    nc.sync.dma_start(
                        kvd.ap()[512 + ft * 128:512 + (ft + 1) * 128,
                                 p0:p0 + PANEL], kvim[:])
                    red = wkp.tile([128, 1], F32, tag="red")
                    nc.vector.tensor_reduce(red[:], kvre[:],
                                            axis=mybir.AxisListType.X,
                                            op=AT.add)
                    nc.gpsimd.tensor_tensor(acc[ft][:], acc[ft][:], red[:],
                                            op=AT.add) if p else \
                        nc.gpsimd.tensor_copy(acc[ft][:], red[:])
                    red2 = wkp.tile([128, 1], F32, tag="red")
                    nc.vector.tensor_reduce(red2[:], kvim[:],
                                            axis=mybir.AxisListType.X,
                                            op=AT.add)
                    nc.gpsimd.tensor_tensor(acc[4 + ft][:], acc[4 + ft][:],
                                            red2[:], op=AT.add) if p else \
                        nc.gpsimd.tensor_copy(acc[4 + ft][:], red2[:])
                # nyquist: fk512, fv512 real rows (separate M=1 groups,
                # partition-0 base everywhere)
                pnk = psN.tile([1, PANEL], F32, tag="nyqk")
                pnv = psN.tile([1, PANEL], F32, tag="nyqv")
                for dp in range(NDP):
                    nc.tensor.matmul(pnk[:], mnyq[dp][:, 0:1], ht[dp][:],
                                     start=(dp == 0),
                                     stop=(dp == NDP - 1 and not has_bias))
                for dp in range(NDP):
                    nc.tensor.matmul(pnv[:], mnyq[dp][:, 1:2], ht[dp][:],
                                     start=(dp == 0),
                                     stop=(dp == NDP - 1 and not has_bias))
                if has_bias:
                    nc.tensor.matmul(pnk[:], bias[:, 4 * 520:4 * 520 + 1],
                                     ones[:], start=False, stop=True)
                    nc.tensor.matmul(pnv[:], bias[:, 4 * 520 + 1:4 * 520 + 2],
                                     ones[:], start=False, stop=True)
                nyk = wkp.tile([1, PANEL], F32, tag="nyk")
                nc.scalar.copy(nyk[:], pnk[:])
                kvn = wkp.tile([1, PANEL], F32, tag="kvn")
                nc.vector.tensor_tensor(kvn[:], nyk[:], pnv[:],
                                        op=AT.mult)
                nc.sync.dma_start(kvd.ap()[1024:1025, p0:p0 + PANEL], kvn[:])
                redn = wkp.tile([1, 1], F32, tag="redn")
                nc.vector.tensor_reduce(redn[:], kvn[:],
                                        axis=mybir.AxisListType.X, op=AT.add)
                if p:
                    nc.gpsimd.tensor_tensor(accn[:], accn[:], redn[:],
                                            op=AT.add)
                else:
                    nc.gpsimd.tensor_copy(accn[:], redn[:])

            for i in range(8):
                nc.sync.dma_start(tot_d.ap()[i * 128:(i + 1) * 128, 0:1],
                                  acc[i][:])
            nc.sync.dma_start(tot_d.ap()[1024:1025, 0:1], accn[:])

    _legalize_waits(nc)
    return nc


def _build_b(has_bias):
    import concourse.bass as bass
    import concourse.mybir as mybir
    import concourse.tile as tile
    F32, F32R = mybir.dt.float32, mybir.dt.float32r
    AT = mybir.AluOpType

    nc = bass.Bass("TRN2", target_bir_lowering=False, debug=False,
                   num_devices=NCORES)
    h_d = nc.dram_tensor("h", [CHUNK, D], F32R, kind="ExternalInput")
    kvd = nc.dram_tensor("kvd", [KVROWS, CHUNK], F32, kind="ExternalInput")
    init_d = nc.dram_tensor("init", [1056, 1], F32, kind="ExternalInput")
    outp_d = nc.dram_tensor("outp", [CHUNK, D], F32, kind="ExternalInput")
    m_d = {nm: nc.dram_tensor(nm, [D, 512], F32R, kind="ExternalInput")
           for nm in ("MqC", "MqS")}
    mnyq_d = nc.dram_tensor("Mnyq", [D, 1], F32R, kind="ExternalInput")
    a_d = nc.dram_tensor("A", [512, D], F32R, kind="ExternalInput")
    b_d = nc.dram_tensor("Bm", [512, D], F32R, kind="ExternalInput")
    a512_d = nc.dram_tensor("A512", [1, D], F32R, kind="ExternalInput")
    identr_d = nc.dram_tensor("identr", [128, 128], F32R, kind="ExternalInput")
    if has_bias:
        bias_d = nc.dram_tensor("biasB", [1, 2 * 520 + 1], F32R,
                                kind="ExternalInput")
        ones_d = nc.dram_tensor("ones", [1, PANEL], F32R, kind="ExternalInput")
    res_d = nc.dram_tensor("res", [CHUNK, D], F32, kind="ExternalOutput")

    with tile.TileContext(nc) as tc:
        with (
            tc.tile_pool(name="const", bufs=1) as cp,
            tc.tile_pool(name="wpool", bufs=1) as wp,
            tc.tile_pool(name="ht", bufs=1) as htp,
            tc.tile_pool(name="hnat", bufs=2) as hnp,
            tc.tile_pool(name="kvp", bufs=3) as kvp,
            tc.tile_pool(name="memp", bufs=10) as memp,
            tc.tile_pool(name="carry", bufs=1) as carp,
            tc.tile_pool(name="work", bufs=3) as wkp,
            tc.tile_pool(name="zpool", bufs=1) as zp,
            tc.tile_pool(name="io", bufs=2) as iop,
            tc.tile_pool(name="psQ", bufs=2, space="PSUM") as psQ,
            tc.tile_pool(name="psN", bufs=1, space="PSUM") as psN,
            tc.tile_pool(name="psV", bufs=2, space="PSUM") as psV,
            tc.tile_pool(name="psT", bufs=2, space="PSUM") as pst,
        ):
            identr = cp.tile([128, 128], F32R, tag="identr")
            nc.sync.dma_start(identr[:], identr_d.ap())
            if has_bias:
                bias = cp.tile([1, 2 * 520 + 1], F32R, tag="bias")
                nc.sync.dma_start(bias[:], bias_d.ap())
                ones = cp.tile([1, PANEL], F32R, tag="ones")
                nc.sync.dma_start(ones[:], ones_d.ap())
            mq = {}
            for nm in ("MqC", "MqS"):
                for dp in range(NDP):
                    t = wp.tile([128, 512], F32R, tag=f"m_{nm}_{dp}")
                    nc.sync.dma_start(
                        t[:], m_d[nm].ap()[dp * 128:(dp + 1) * 128, :])
                    mq[(nm, dp)] = t
            mnyq = []
            for dp in range(NDP):
                t = cp.tile([128, 1], F32R, tag=f"mnyq_{dp}")
                nc.sync.dma_start(t[:], mnyq_d.ap()[dp * 128:(dp + 1) * 128, :])
                mnyq.append(t)
            asb, bsb = [], []
            for ftt in range(FT):
                ta = wp.tile([128, D], F32R, tag=f"a_{ftt}")
                nc.sync.dma_start(ta[:], a_d.ap()[ftt * 128:(ftt + 1) * 128, :])
                asb.append(ta)
                tb = wp.tile([128, D], F32R, tag=f"b_{ftt}")
                nc.sync.dma_start(tb[:], b_d.ap()[ftt * 128:(ftt + 1) * 128, :])
                bsb.append(tb)
            a512 = cp.tile([1, D], F32R, tag="a512")
            nc.sync.dma_start(a512[:], a512_d.ap())

            # scan carries: init columns from DRAM (host prefix)
            carry = []
            for i in range(9):
                t = carp.tile([128, 1], F32, tag=f"car_{i}")
                r0 = i * 128 if i < 8 else 1024
                rows = 128 if i < 8 else 1
                nc.sync.dma_start(t[:rows, :], init_d.ap()[r0:r0 + rows, 0:1])
                carry.append(t)

            for blk in range(NPANEL):
                p0 = blk * PANEL
                ht = _make_ht(nc, tc, htp, hnp, pst, h_d, identr, p0)
                # mem for this block: scan kv with chained carry
                mems = []
                for i in range(9):
                    rows = 128 if i < 8 else 1
                    r0 = i * 128 if i < 8 else 1024
                    kvt = kvp.tile([128, PANEL], F32, tag="kvt")
                    nc.sync.dma_start(kvt[:rows, :],
                                      kvd.ap()[r0:r0 + rows, p0:p0 + PANEL])
                    mt = memp.tile([128, PANEL], F32, tag="memt")
                    nc.vector.tensor_tensor_scan(
                        mt[:rows, :], kvt[:rows, :], kvt[:rows, :],
                        carry[i][:rows, :], op0=AT.add, op1=AT.bypass)
                    nc.gpsimd.tensor_copy(carry[i][:rows, :],
                                          mt[:rows, PANEL - 1:PANEL])
                    mems.append(mt)
                zre, zim = [], []
                for ft in range(FT):
                    pq = {}
                    for i, nm in enumerate(("MqC", "MqS")):
                        pt = psQ.tile([128, PANEL], F32, tag="fq")
                        for dp in range(NDP):
                            nc.tensor.matmul(
                                pt[:], mq[(nm, dp)][:, ft * 128:(ft + 1) * 128],
                                ht[dp][:], start=(dp == 0),
                                stop=(dp == NDP - 1 and not has_bias))
                        if has_bias:
                            nc.tensor.matmul(
                                pt[:],
                                bias[:, i * 520 + ft * 128:i * 520 + (ft + 1) * 128],
                                ones[:], start=False, stop=True)
                        pq[nm] = pt
                    t1 = wkp.tile([128, PANEL], F32, tag="t1")
                    t2 = wkp.tile([128, PANEL], F32, tag="t2")
                    zr = zp.tile([128, PANEL], F32R, tag=f"zre_{ft}")
                    zi = zp.tile([128, PANEL], F32R, tag=f"zim_{ft}")
                    nc.vector.tensor_tensor(t1[:], mems[ft][:], pq["MqC"][:],
                                            op=AT.mult)
                    nc.vector.tensor_tensor(t2[:], mems[4 + ft][:],
                                            pq["MqS"][:], op=AT.mult)
                    nc.vector.tensor_tensor(zr[:], t1[:], t2[:],
                                            op=AT.subtract)
                    nc.vector.tensor_tensor(t1[:], mems[ft][:], pq["MqS"][:],
                                            op=AT.mult)
                    nc.vector.tensor_tensor(t2[:], mems[4 + ft][:],
                                            pq["MqC"][:], op=AT.mult)
                    nc.vector.tensor_tensor(zi[:], t1[:], t2[:], op=AT.add)
                    zre.append(zr)
                    zim.append(zi)
                # nyquist fq
                pn = psN.tile([1, PANEL], F32, tag="fqnyq")
                for dp in range(NDP):
                    nc.tensor.matmul(pn[:], mnyq[dp][:], ht[dp][:],
                                     start=(dp == 0),
                                     stop=(dp == NDP - 1 and not has_bias))
                if has_bias:
                    nc.tensor.matmul(pn[:], bias[:, 2 * 520:2 * 520 + 1],
                                     ones[:], start=False, stop=True)
                znyq = zp.tile([1, PANEL], F32R, tag="znyq")
                nc.vector.tensor_tensor(znyq[:], mems[8][0:1, :], pn[:],
                                        op=AT.mult)

                for sub in range(PANEL // 128):
                    ob = iop.tile([128, D], F32, tag="ob")
                    nc.sync.dma_start(
                        ob[:], outp_d.ap()[p0 + sub * 128:p0 + (sub + 1) * 128, :])
                    rs = iop.tile([128, D], F32, tag="rs")
                    s0, s1 = sub * 128, (sub + 1) * 128
                    for half in range(2):
                        pv = psV.tile([128, 512], F32, tag="pv")
                        d0, d1 = half * 512, (half + 1) * 512
                        for ft in range(FT):
                            nc.tensor.matmul(pv[:], zre[ft][:, s0:s1],
                                             asb[ft][:, d0:d1],
                                             start=(ft == 0), stop=False)
                        for ft in range(FT):
                            nc.tensor.matmul(pv[:], zim[ft][:, s0:s1],
                                             bsb[ft][:, d0:d1],
                                             start=False, stop=False)
                        nc.tensor.matmul(pv[:], znyq[:, s0:s1],
                                         a512[:, d0:d1],
                                         start=False, stop=True)
                        nc.vector.tensor_tensor(rs[:, d0:d1], pv[:],
                                                ob[:, d0:d1], op=AT.add)
                    nc.sync.dma_start(
                        res_d.ap()[p0 + sub * 128:p0 + (sub + 1) * 128, :],
                        rs[:])

    _legalize_waits(nc)
    return nc


def _programs(has_bias):
    key = ("ab", has_bias)
    if key not in _cache:
        _cache[key] = (_build_a(has_bias), _build_b(has_bias))
    return _cache[key]


def kernel(output, hidden_states, Wq, bq, Wk, bk, Wv, bv, gate, _trace=False):
    from concourse import bass_utils

    output = np.asarray(output, dtype=np.float32)
    hidden = np.asarray(hidden_states, dtype=np.float32)
    cst = _host_constants(
        np.asarray(Wq, np.float32), np.asarray(bq, np.float32),
        np.asarray(Wk, np.float32), np.asarray(bk, np.float32),
        np.asarray(Wv, np.float32), np.asarray(bv, np.float32),
        np.asarray(gate, np.float32))
    has_bias = bool(np.any(cst["bias"]))
    nca, ncb = _programs(has_bias)

    ac = np.ascontiguousarray
    ident = np.eye(128, dtype=np.float32)
    sharedA = {
        "MkC": ac(cst["MkC"][:, :512]), "MkS": ac(cst["MkS"][:, :512]),
        "MvC": ac(cst["MvC"][:, :512]), "MvS": ac(cst["MvS"][:, :512]),
        "Mnyq": ac(np.stack([cst["MkC"][:, 512], cst["MvC"][:, 512]], axis=1)),
        "identr": ident,
    }
    if has_bias:
        ba = np.zeros((1, 4 * 520 + 2), np.float32)
        for i in range(4):
            ba[0, i * 520:i * 520 + 520] = cst["bias"][i]
        ba[0, 4 * 520 + 0] = cst["bias"][0][512]
        ba[0, 4 * 520 + 1] = cst["bias"][2][512]
        sharedA["biasA"] = ba
        sharedA["ones"] = np.ones((1, PANEL), np.float32)

    chunks = []
    for c in range(NCORES):
        b, j = c // 4, c % 4
        chunks.append((b, j))

    in_a = []
    for (b, j) in chunks:
        im = dict(sharedA)
        im["h"] = ac(hidden[b, j * CHUNK:(j + 1) * CHUNK, :])
        in_a.append(im)
    res_a = bass_utils.run_bass_kernel_spmd(
        nca, in_a, core_ids=list(range(NCORES)), trace=_trace)

    # host: causal prefix over chunk totals
    totals = np.stack([res_a.results[c]["totals"][:, 0] for c in range(NCORES)])
    inits = []
    for c, (b, j) in enumerate(chunks):
        p = np.zeros((1056, 1), np.float32)
        for c2, (b2, j2) in enumerate(chunks):
            if b2 == b and j2 < j:
                p[:, 0] += totals[c2]
        inits.append(p)

    sharedB = {
        "MqC": ac(cst["MqC"][:, :512]), "MqS": ac(cst["MqS"][:, :512]),
        "Mnyq": ac(cst["MqC"][:, 512:513]),
        "A": ac(cst["A"][:512, :]), "Bm": ac(cst["Bm"][:512, :]),
        "A512": ac(cst["A"][512:513, :]),
        "identr": ident,
    }
    if has_bias:
        bb = np.zeros((1, 2 * 520 + 1), np.float32)
        bb[0, 0:520] = cst["bias"][4]
        bb[0, 520:1040] = cst["bias"][5]
        bb[0, 2 * 520] = cst["bias"][4][512]
        sharedB["biasB"] = bb
        sharedB["ones"] = np.ones((1, PANEL), np.float32)

    in_b = []
    for c, (b, j) in enumerate(chunks):
        im = dict(sharedB)
        im["h"] = in_a[c]["h"]
        im["kvd"] = res_a.results[c]["kvd"]
        im["init"] = inits[c]
        im["outp"] = ac(output[b, j * CHUNK:(j + 1) * CHUNK, :])
        in_b.append(im)
    res_b = bass_utils.run_bass_kernel_spmd(
        ncb, in_b, core_ids=list(range(NCORES)), trace=_trace)

    out = np.empty((B, S, D), dtype=np.float32)
    for c, (b, j) in enumerate(chunks):
        out[b, j * CHUNK:(j + 1) * CHUNK, :] = res_b.results[c]["res"]
    if _trace:
        kernel._last = (res_a, res_b)
    return out


# revision 12
# speedup vs baseline: 26004.0969x; 26004.0969x over previous
"""Trainium2 Bass kernel for nn_HRRAdaptedAttention (B=2, S=8192, D=1024).

out = output + gate * irfft(cumsum_s(rfft(k)*rfft(v)) * conj(rfft(q))),
q/k/v = hidden @ W.T + b.

Sharding: (batch, seq) -> 8 chunks of 2048 positions, one per core.
The rfft/irfft are folded into the projection weights on the host
(fk = h @ (Wk.T @ C) etc.), so everything on device is fp32r matmuls,
elementwise complex arithmetic, and a per-frequency cumsum over the
sequence axis (tensor_tensor_scan, [freq->partitions, seq->free] layout).

Launch 1 (per core): h^T -> fk,fv -> kv = fk*fv -> kv chunk to DRAM,
plus per-frequency chunk totals.
Host: 8x[1025] exclusive prefix over chunk totals (causal carry).
Launch 2: scan(kv, initial=prefix) -> mem; fq; Z = mem*conj(fq);
values = Z @ [A;B] (gate folded); res = output + values.
"""

import numpy as np

B, S, D = 2, 8192, 1024
F = 513
NCORES = 8
CHUNK = 2048
PANEL = 512
NPANEL = CHUNK // PANEL
FT = 4                   # 128-row freq tiles f=0..511; f=512 handled apart
NDP = 8
KVROWS = 1152            # kv dram rows: 512 re + 512 im + kvnyq + fqnyq

_cache = {}


def _host_constants(Wq, bq, Wk, bk, Wv, bv, gate):
    d = np.arange(D, dtype=np.float64)
    f = np.arange(F, dtype=np.float64)
    ang = 2.0 * np.pi * np.outer(d, f) / D
    C = np.cos(ang)
    Sm = -np.sin(ang)

    def fold(W, sign_s=1.0):
        Wt = W.T.astype(np.float64)
        return (Wt @ C).astype(np.float32), (sign_s * (Wt @ Sm)).astype(np.float32)

    MkC, MkS = fold(Wk)
    MvC, MvS = fold(Wv)
    MqC, MqS = fold(Wq, sign_s=-1.0)          # conj(fq) folded

    g = float(np.asarray(gate).reshape(-1)[0])
    w = np.full(F, 2.0)
    w[0] = 1.0
    w[512] = 1.0
    scale = (w * g / D)[:, None]
    A = (scale * C.T).astype(np.float32)       # [F, D] coeff for Zre
    Bm = (scale * Sm.T).astype(np.float32)     # [F, D] coeff for Zim

    bk64, bv64, bq64 = (x.astype(np.float64) for x in (bk, bv, bq))
    bias = np.zeros((6, 520), dtype=np.float32)
    bias[0, :F] = (bk64 @ C).astype(np.float32)
    bias[1, :F] = (bk64 @ Sm).astype(np.float32)
    bias[2, :F] = (bv64 @ C).astype(np.float32)
    bias[3, :F] = (bv64 @ Sm).astype(np.float32)
    bias[4, :F] = (bq64 @ C).astype(np.float32)
    bias[5, :F] = (-(bq64 @ Sm)).astype(np.float32)
    return dict(MkC=MkC, MkS=MkS, MvC=MvC, MvS=MvS, MqC=MqC, MqS=MqS,
                A=A, Bm=Bm, bias=bias)


_WAIT_EXEMPT = {
    "InstNoOp", "InstEventSemaphore", "InstUnconditionalBranch",
    "InstRegisterMove", "InstCall", "InstISA",
}


def _legalize_waits(nc, max_waits=1):
    """TRN2 instruction structs hold one sync-wait command; move extra waits
    onto same-engine nops inserted just before the instruction."""
    import bass_rust
    import concourse.mybir as mybir
    ctr = 0
    for fn in nc.m.functions:
        for blk in fn.blocks:
            new = []
            for inst in blk.instructions:
                if (type(inst).__name__ not in _WAIT_EXEMPT
                        and inst.sync_info is not None):
                    waits = list(inst.sync_info.on_wait)
                    if len(waits) > max_waits:
                        for w in waits[:-max_waits]:
                            nop = mybir.InstNoOp(
                                name=f"I-lglnop-{ctr}", ins=[], outs=[])
                            ctr += 1
                            nop.engine = inst.engine
                            nop.sync_info = bass_rust.SyncInfo(
                                on_wait=[w], on_update=[])
                            new.append(nop)
                        inst.sync_info = bass_rust.SyncInfo(
                            on_wait=waits[-max_waits:],
                            on_update=inst.sync_info.on_update)
                new.append(inst)
            blk.instructions = new


def _make_ht(nc, tc, htp, hnp, pst, h_d, identr, p0):
    """Load h[p0:p0+PANEL] and emit h^T tiles [128d, PANEL] (fp32r)."""
    import concourse.mybir as mybir
    F32R = mybir.dt.float32r
    ht = [htp.tile([128, PANEL], F32R, tag=f"ht_{dp}", name=f"ht_{dp}") for dp in range(NDP)]
    for st in range(PANEL // 128):
        hn = hnp.tile([128, D], F32R, tag="hn")
        nc.sync.dma_start(hn[:], h_d.ap()[p0 + st * 128:p0 + (st + 1) * 128, :])
        for dp in range(NDP):
            tp = pst.tile([128, 128], F32R, tag="trps")
            nc.tensor.transpose(tp[:], hn[:, dp * 128:(dp + 1) * 128],
                                identr[:])
            nc.scalar.copy(ht[dp][:, st * 128:(st + 1) * 128], tp[:])
    return ht


def _build_a(has_bias):
    import concourse.bass as bass
    import concourse.mybir as mybir
    import concourse.tile as tile
    F32, F32R = mybir.dt.float32, mybir.dt.float32r
    AT = mybir.AluOpType

    nc = bass.Bass("TRN2", target_bir_lowering=False, debug=False,
                   num_devices=NCORES)
    h_d = nc.dram_tensor("h", [CHUNK, D], F32R, kind="ExternalInput")
    m_d = {nm: nc.dram_tensor(nm, [D, 512], F32R, kind="ExternalInput")
           for nm in ("MkC", "MkS", "MvC", "MvS")}
    mnyq_d = nc.dram_tensor("Mnyq", [D, 2], F32R, kind="ExternalInput")
    identr_d = nc.dram_tensor("identr", [128, 128], F32R, kind="ExternalInput")
    if has_bias:
        bias_d = nc.dram_tensor("biasA", [1, 4 * 520 + 2], F32R,
                                kind="ExternalInput")
        ones_d = nc.dram_tensor("ones", [1, PANEL], F32R, kind="ExternalInput")
    kvd = nc.dram_tensor("kvd", [KVROWS, CHUNK], F32, kind="ExternalOutput")
    tot_d = nc.dram_tensor("totals", [1056, 1], F32, kind="ExternalOutput")
    htd = nc.dram_tensor("htd", [D, CHUNK], F32R, kind="ExternalOutput")

    with tile.TileContext(nc) as tc:
        with (
            tc.tile_pool(name="const", bufs=1) as cp,
            tc.tile_pool(name="wpool", bufs=1) as wp,
            tc.tile_pool(name="ht", bufs=2) as htp,
            tc.tile_pool(name="hnat", bufs=3) as hnp,
            tc.tile_pool(name="work", bufs=3) as wkp,
            tc.tile_pool(name="acc", bufs=1) as accp,
            tc.tile_pool(name="psA", bufs=4, space="PSUM") as psA,
            tc.tile_pool(name="psN", bufs=1, space="PSUM") as psN,
            tc.tile_pool(name="psT", bufs=2, space="PSUM") as pst,
        ):
            identr = cp.tile([128, 128], F32R, tag="identr")
            nc.sync.dma_start(identr[:], identr_d.ap())
            if has_bias:
                bias = cp.tile([1, 4 * 520 + 2], F32R, tag="bias")
                nc.sync.dma_start(bias[:], bias_d.ap())
                ones = cp.tile([1, PANEL], F32R, tag="ones")
                nc.sync.dma_start(ones[:], ones_d.ap())
            mkv = {}
            for nm in ("MkC", "MkS", "MvC", "MvS"):
                for dp in range(NDP):
                    t = wp.tile([128, 512], F32R, tag=f"m_{nm}_{dp}")
                    nc.sync.dma_start(
                        t[:], m_d[nm].ap()[dp * 128:(dp + 1) * 128, :])
                    mkv[(nm, dp)] = t
            mnyq = []
            for dp in range(NDP):
                t = cp.tile([128, 2], F32R, tag=f"mnyq_{dp}")
                nc.sync.dma_start(t[:], mnyq_d.ap()[dp * 128:(dp + 1) * 128, :])
                mnyq.append(t)

            acc = {i: accp.tile([128, 1], F32, tag=f"acc_{i}", name=f"acc_{i}") for i in range(8)}
            accn = accp.tile([1, 1], F32, tag="acc_n")

            for p in range(NPANEL):
                p0 = p * PANEL
                ht = _make_ht(nc, tc, htp, hnp, pst, h_d, identr, p0)
                for dp in range(NDP):
                    nc.sync.dma_start(
                        htd.ap()[dp * 128:(dp + 1) * 128, p0:p0 + PANEL],
                        ht[dp][:])
                for ft in range(FT):
                    ps = {}
                    for i, nm in enumerate(("MkC", "MkS", "MvC", "MvS")):
                        pt = psA.tile([128, PANEL], F32, tag="fwd")
                        for dp in range(NDP):
                            nc.tensor.matmul(
                                pt[:], mkv[(nm, dp)][:, ft * 128:(ft + 1) * 128],
                                ht[dp][:], start=(dp == 0),
                                stop=(dp == NDP - 1 and not has_bias))
                        if has_bias:
                            nc.tensor.matmul(
                                pt[:],
                                bias[:, i * 520 + ft * 128:i * 520 + (ft + 1) * 128],
                                ones[:], start=False, stop=True)
                        ps[nm] = pt
                    fkre = wkp.tile([128, PANEL], F32, tag="fkre")
                    fkim = wkp.tile([128, PANEL], F32, tag="fkim")
                    nc.scalar.copy(fkre[:], ps["MkC"][:])
                    nc.scalar.copy(fkim[:], ps["MkS"][:])
                    t1 = wkp.tile([128, PANEL], F32, tag="t1")
                    t2 = wkp.tile([128, PANEL], F32, tag="t2")
                    kvre = wkp.tile([128, PANEL], F32, tag="kvre")
                    kvim = wkp.tile([128, PANEL], F32, tag="kvim")
                    nc.vector.tensor_tensor(t1[:], fkre[:], ps["MvC"][:],
                                            op=AT.mult)
                    nc.vector.tensor_tensor(t2[:], fkim[:], ps["MvS"][:],
                                            op=AT.mult)
                    nc.vector.tensor_tensor(kvre[:], t1[:], t2[:],
                                            op=AT.subtract)
                    nc.vector.tensor_tensor(t1[:], fkre[:], ps["MvS"][:],
                                            op=AT.mult)
                    nc.vector.tensor_tensor(t2[:], fkim[:], ps["MvC"][:],
                                            op=AT.mult)
                    nc.vector.tensor_tensor(kvim[:], t1[:], t2[:], op=AT.add)
                    nc.sync.dma_start(
                        kvd.ap()[ft * 128:(ft + 1) * 128, p0:p0 + PANEL],
                        kvre[:])
                    nc.sync.dma_start(
                        kvd.ap()[512 + ft * 128:512 + (ft + 1) * 128,
                                 p0:p0 + PANEL], kvim[:])
                    red = wkp.tile([128, 1], F32, tag="red")
                    nc.vector.tensor_reduce(red[:], kvre[:],
                                            axis=mybir.AxisListType.X,
                                            op=AT.add)
                    nc.gpsimd.tensor_tensor(acc[ft][:], acc[ft][:], red[:],
                                            op=AT.add) if p else \
                        nc.gpsimd.tensor_copy(acc[ft][:], red[:])
                    red2 = wkp.tile([128, 1], F32, tag="red")
                    nc.vector.tensor_reduce(red2[:], kvim[:],
                                            axis=mybir.AxisListType.X,
                                            op=AT.add)
                    nc.gpsimd.tensor_tensor(acc[4 + ft][:], acc[4 + ft][:],
                                            red2[:], op=AT.add) if p else \
                        nc.gpsimd.tensor_copy(acc[4 + ft][:], red2[:])
                # nyquist: fk512, fv512 real rows (separate M=1 groups,
                # partition-0 base everywhere)
                pnk = psN.tile([1, PANEL], F32, tag="nyqk")
                pnv = psN.tile([1, PANEL], F32, tag="nyqv")
                for dp in range(NDP):
                    nc.tensor.matmul(pnk[:], mnyq[dp][:, 0:1], ht[dp][:],
                                     start=(dp == 0),
                                     stop=(dp == NDP - 1 and not has_bias))
                for dp in range(NDP):
                    nc.tensor.matmul(pnv[:], mnyq[dp][:, 1:2], ht[dp][:],
                                     start=(dp == 0),
                                     stop=(dp == NDP - 1 and not has_bias))
                if has_bias:
                    nc.tensor.matmul(pnk[:], bias[:, 4 * 520:4 * 520 + 1],
                                     ones[:], start=False, stop=True)
                    nc.tensor.matmul(pnv[:], bias[:, 4 * 520 + 1:4 * 520 + 2],
                                     ones[:], start=False, stop=True)
                nyk = wkp.tile([1, PANEL], F32, tag="nyk")
                nc.scalar.copy(nyk[:], pnk[:])
                kvn = wkp.tile([1, PANEL], F32, tag="kvn")
                nc.vector.tensor_tensor(kvn[:], nyk[:], pnv[:],
                                        op=AT.mult)
                nc.sync.dma_start(kvd.ap()[1024:1025, p0:p0 + PANEL], kvn[:])
                redn = wkp.tile([1, 1], F32, tag="redn")
                nc.vector.tensor_reduce(redn[:], kvn[:],
                                        axis=mybir.AxisListType.X, op=AT.add)
                if p:
                    nc.gpsimd.tensor_tensor(accn[:], accn[:], redn[:],
                                            op=AT.add)
                else:
                    nc.gpsimd.tensor_copy(accn[:], redn[:])

            for i in range(8):
                nc.sync.dma_start(tot_d.ap()[i * 128:(i + 1) * 128, 0:1],
                                  acc[i][:])
            nc.sync.dma_start(tot_d.ap()[1024:1025, 0:1], accn[:])

    _legalize_waits(nc)
    return nc


def _build_b(has_bias):
    import concourse.bass as bass
    import concourse.mybir as mybir
    import concourse.tile as tile
    F32, F32R = mybir.dt.float32, mybir.dt.float32r
    AT = mybir.AluOpType

    nc = bass.Bass("TRN2", target_bir_lowering=False, debug=False,
                   num_devices=NCORES)
    htd = nc.dram_tensor("htd", [D, CHUNK], F32R, kind="ExternalInput")
    kvd = nc.dram_tensor("kvd", [KVROWS, CHUNK], F32, kind="ExternalInput")
    init_d = nc.dram_tensor("init", [1056, 1], F32, kind="ExternalInput")
    outp_d = nc.dram_tensor("outp", [CHUNK, D], F32, kind="ExternalInput")
    m_d = {nm: nc.dram_tensor(nm, [D, 512], F32R, kind="ExternalInput")
           for nm in ("MqC", "MqS")}
    mnyq_d = nc.dram_tensor("Mnyq", [D, 1], F32R, kind="ExternalInput")
    a_d = nc.dram_tensor("A", [512, D], F32R, kind="ExternalInput")
    b_d = nc.dram_tensor("Bm", [512, D], F32R, kind="ExternalInput")
    a512_d = nc.dram_tensor("A512", [1, D], F32R, kind="ExternalInput")
    if has_bias:
        bias_d = nc.dram_tensor("biasB", [1, 2 * 520 + 1], F32R,
                                kind="ExternalInput")
        ones_d = nc.dram_tensor("ones", [1, PANEL], F32R, kind="ExternalInput")
    res_d = nc.dram_tensor("res", [CHUNK, D], F32, kind="ExternalOutput")

    with tile.TileContext(nc) as tc:
        with (
            tc.tile_pool(name="const", bufs=1) as cp,
            tc.tile_pool(name="wpool", bufs=1) as wp,
            tc.tile_pool(name="ht", bufs=2) as htp,
            tc.tile_pool(name="kvp", bufs=3) as kvp,
            tc.tile_pool(name="memp", bufs=10) as memp,
            tc.tile_pool(name="carry", bufs=1) as carp,
            tc.tile_pool(name="work", bufs=3) as wkp,
            tc.tile_pool(name="zpool", bufs=1) as zp,
            tc.tile_pool(name="io", bufs=2) as iop,
            tc.tile_pool(name="psQ", bufs=4, space="PSUM") as psQ,
            tc.tile_pool(name="psN", bufs=1, space="PSUM") as psN,
            tc.tile_pool(name="psV", bufs=3, space="PSUM") as psV,
        ):
            if has_bias:
                bias = cp.tile([1, 2 * 520 + 1], F32R, tag="bias")
                nc.sync.dma_start(bias[:], bias_d.ap())
                ones = cp.tile([1, PANEL], F32R, tag="ones")
                nc.sync.dma_start(ones[:], ones_d.ap())
            mq = {}
            for nm in ("MqC", "MqS"):
                for dp in range(NDP):
                    t = wp.tile([128, 512], F32R, tag=f"m_{nm}_{dp}")
                    nc.sync.dma_start(
                        t[:], m_d[nm].ap()[dp * 128:(dp + 1) * 128, :])
                    mq[(nm, dp)] = t
            mnyq = []
            for dp in range(NDP):
                t = cp.tile([128, 1], F32R, tag=f"mnyq_{dp}")
                nc.sync.dma_start(t[:], mnyq_d.ap()[dp * 128:(dp + 1) * 128, :])
                mnyq.append(t)
            asb, bsb = [], []
            for ftt in range(FT):
                ta = wp.tile([128, D], F32R, tag=f"a_{ftt}")
                nc.sync.dma_start(ta[:], a_d.ap()[ftt * 128:(ftt + 1) * 128, :])
                asb.append(ta)
                tb = wp.tile([128, D], F32R, tag=f"b_{ftt}")
                nc.sync.dma_start(tb[:], b_d.ap()[ftt * 128:(ftt + 1) * 128, :])
                bsb.append(tb)
            a512 = cp.tile([1, D], F32R, tag="a512")
            nc.sync.dma_start(a512[:], a512_d.ap())

            # scan carries: init columns from DRAM (host prefix)
            carry = []
            for i in range(9):
                t = carp.tile([128, 1], F32, tag=f"car_{i}")
                r0 = i * 128 if i < 8 else 1024
                rows = 128 if i < 8 else 1
                nc.sync.dma_start(t[:rows, :], init_d.ap()[r0:r0 + rows, 0:1])
                carry.append(t)

            for blk in range(NPANEL):
                p0 = blk * PANEL
                ht = [htp.tile([128, PANEL], F32R, tag=f"ht_{dp}",
                               name=f"ht_{blk}_{dp}") for dp in range(NDP)]
                for dp in range(NDP):
                    nc.sync.dma_start(
                        ht[dp][:], htd.ap()[dp * 128:(dp + 1) * 128,
                                            p0:p0 + PANEL])
                # mem for this block: scan kv with chained carry
                mems = []
                for i in range(9):
                    rows = 128 if i < 8 else 1
                    r0 = i * 128 if i < 8 else 1024
                    kvt = kvp.tile([128, PANEL], F32, tag="kvt")
                    nc.sync.dma_start(kvt[:rows, :],
                                      kvd.ap()[r0:r0 + rows, p0:p0 + PANEL])
                    mt = memp.tile([128, PANEL], F32, tag="memt")
                    nc.vector.tensor_tensor_scan(
                        mt[:rows, :], kvt[:rows, :], kvt[:rows, :],
                        carry[i][:rows, :], op0=AT.add, op1=AT.bypass)
                    nc.vector.tensor_copy(carry[i][:rows, :],
                                          mt[:rows, PANEL - 1:PANEL])
                    mems.append(mt)
                zre, zim = [], []
                for ft in range(FT):
                    pq = {}
                    for i, nm in enumerate(("MqC", "MqS")):
                        pt = psQ.tile([128, PANEL], F32, tag="fq")
                        for dp in range(NDP):
                            nc.tensor.matmul(
                                pt[:], mq[(nm, dp)][:, ft * 128:(ft + 1) * 128],
                                ht[dp][:], start=(dp == 0),
                                stop=(dp == NDP - 1 and not has_bias))
                        if has_bias:
                            nc.tensor.matmul(
                                pt[:],
                                bias[:, i * 520 + ft * 128:i * 520 + (ft + 1) * 128],
                                ones[:], start=False, stop=True)
                        pq[nm] = pt
                    t1 = wkp.tile([128, PANEL], F32, tag="t1")
                    t2 = wkp.tile([128, PANEL], F32, tag="t2")
                    zr = zp.tile([128, PANEL], F32R, tag=f"zre_{ft}")
                    zi = zp.tile([128, PANEL], F32R, tag=f"zim_{ft}")
                    nc.vector.tensor_tensor(t1[:], mems[ft][:], pq["MqC"][:],
                                            op=AT.mult)
                    nc.vector.tensor_tensor(t2[:], mems[4 + ft][:],
                                            pq["MqS"][:], op=AT.mult)
                    nc.vector.tensor_tensor(zr[:], t1[:], t2[:],
                                            op=AT.subtract)
                    nc.vector.tensor_tensor(t1[:], mems[ft][:], pq["MqS"][:],
                                            op=AT.mult)
                    nc.vector.tensor_tensor(t2[:], mems[4 + ft][:],
                                            pq["MqC"][:], op=AT.mult)
                    nc.vector.tensor_tensor(zi[:], t1[:], t2[:], op=AT.add)
                    zre.append(zr)
                    zim.append(zi)
                # nyquist fq
                pn = psN.tile([1, PANEL], F32, tag="fqnyq")
                for dp in range(NDP):
                    nc.tensor.matmul(pn[:], mnyq[dp][:], ht[dp][:],
                                     start=(dp == 0),
                                     stop=(dp == NDP - 1 and not has_bias))
                if has_bias:
                    nc.tensor.matmul(pn[:], bias[:, 2 * 520:2 * 520 + 1],
                                     ones[:], start=False, stop=True)
                znyq = zp.tile([1, PANEL], F32R, tag="znyq")
                nc.vector.tensor_tensor(znyq[:], mems[8][0:1, :], pn[:],
                                        op=AT.mult)

                for sub in range(PANEL // 128):
                    ob = iop.tile([128, D], F32, tag="ob")
                    nc.sync.dma_start(
                        ob[:], outp_d.ap()[p0 + sub * 128:p0 + (sub + 1) * 128, :])
                    rs = iop.tile([128, D], F32, tag="rs")
                    s0, s1 = sub * 128, (sub + 1) * 128
                    for half in range(2):
                        pv = psV.tile([128, 512], F32, tag="pv")
                        d0, d1 = half * 512, (half + 1) * 512
                        for ft in range(FT):
                            nc.tensor.matmul(pv[:], zre[ft][:, s0:s1],
                                             asb[ft][:, d0:d1],
                                             start=(ft == 0), stop=False)
                        for ft in range(FT):
                            nc.tensor.matmul(pv[:], zim[ft][:, s0:s1],
                                             bsb[ft][:, d0:d1],
                                             start=False, stop=False)
                        nc.tensor.matmul(pv[:], znyq[:, s0:s1],
                                         a512[:, d0:d1],
                                         start=False, stop=True)
                        nc.vector.tensor_tensor(rs[:, d0:d1], pv[:],
                                                ob[:, d0:d1], op=AT.add)
                    nc.sync.dma_start(
                        res_d.ap()[p0 + sub * 128:p0 + (sub + 1) * 128, :],
                        rs[:])

    _legalize_waits(nc)
    return nc


def _programs(has_bias):
    key = ("ab", has_bias)
    if key not in _cache:
        _cache[key] = (_build_a(has_bias), _build_b(has_bias))
    return _cache[key]


def kernel(output, hidden_states, Wq, bq, Wk, bk, Wv, bv, gate, _trace=False):
    from concourse import bass_utils

    output = np.asarray(output, dtype=np.float32)
    hidden = np.asarray(hidden_states, dtype=np.float32)
    cst = _host_constants(
        np.asarray(Wq, np.float32), np.asarray(bq, np.float32),
        np.asarray(Wk, np.float32), np.asarray(bk, np.float32),
        np.asarray(Wv, np.float32), np.asarray(bv, np.float32),
        np.asarray(gate, np.float32))
    has_bias = bool(np.any(cst["bias"]))
    nca, ncb = _programs(has_bias)

    ac = np.ascontiguousarray
    ident = np.eye(128, dtype=np.float32)
    sharedA = {
        "MkC": ac(cst["MkC"][:, :512]), "MkS": ac(cst["MkS"][:, :512]),
        "MvC": ac(cst["MvC"][:, :512]), "MvS": ac(cst["MvS"][:, :512]),
        "Mnyq": ac(np.stack([cst["MkC"][:, 512], cst["MvC"][:, 512]], axis=1)),
        "identr": ident,
    }
    if has_bias:
        ba = np.zeros((1, 4 * 520 + 2), np.float32)
        for i in range(4):
            ba[0, i * 520:i * 520 + 520] = cst["bias"][i]
        ba[0, 4 * 520 + 0] = cst["bias"][0][512]
        ba[0, 4 * 520 + 1] = cst["bias"][2][512]
        sharedA["biasA"] = ba
        sharedA["ones"] = np.ones((1, PANEL), np.float32)

    chunks = []
    for c in range(NCORES):
        b, j = c // 4, c % 4
        chunks.append((b, j))

    in_a = []
    for (b, j) in chunks:
        im = dict(sharedA)
        im["h"] = ac(hidden[b, j * CHUNK:(j + 1) * CHUNK, :])
        in_a.append(im)
    res_a = bass_utils.run_bass_kernel_spmd(
        nca, in_a, core_ids=list(range(NCORES)), trace=_trace)

    # host: causal prefix over chunk totals
    totals = np.stack([res_a.results[c]["totals"][:, 0] for c in range(NCORES)])
    inits = []
    for c, (b, j) in enumerate(chunks):
        p = np.zeros((1056, 1), np.float32)
        for c2, (b2, j2) in enumerate(chunks):
            if b2 == b and j2 < j:
                p[:, 0] += totals[c2]
        inits.append(p)

    sharedB = {
        "MqC": ac(cst["MqC"][:, :512]), "MqS": ac(cst["MqS"][:, :512]),
        "Mnyq": ac(cst["MqC"][:, 512:513]),
        "A": ac(cst["A"][:512, :]), "Bm": ac(cst["Bm"][:512, :]),
        "A512": ac(cst["A"][512:513, :]),
    }
    if has_bias:
        bb = np.zeros((1, 2 * 520 + 1), np.float32)
        bb[0, 0:520] = cst["bias"][4]
        bb[0, 520:1040] = cst["bias"][5]
        bb[0, 2 * 520] = cst["bias"][4][512]
        sharedB["biasB"] = bb
        sharedB["ones"] = np.ones((1, PANEL), np.float32)

    in_b = []
    for c, (b, j) in enumerate(chunks):
        im = dict(sharedB)
        im["htd"] = res_a.results[c]["htd"]
        im["kvd"] = res_a.results[c]["kvd"]
        im["init"] = inits[c]
        im["outp"] = ac(output[b, j * CHUNK:(j + 1) * CHUNK, :])
        in_b.append(im)
    res_b = bass_utils.run_bass_kernel_spmd(
        ncb, in_b, core_ids=list(range(NCORES)), trace=_trace)

    out = np.empty((B, S, D), dtype=np.float32)
    for c, (b, j) in enumerate(chunks):
        out[b, j * CHUNK:(j + 1) * CHUNK, :] = res_b.results[c]["res"]
    if _trace:
        kernel._last = (res_a, res_b)
    return out
